# revision 1
# baseline (speedup 1.0000x reference)
"""2-layer GAT on 8 Trainium2 NeuronCores (Bass/Tile).

Strategy (edge-parallel per sharding hint, node-per-partition layout):
- Nodes are assigned round-robin by degree rank to the 8 cores, so every
  core sees an almost identical degree profile (uniform SPMD program).
- Per core, virtual node slots are grouped 128 per "group"; a group's
  edges live in a [128 partitions, NCH_g columns] grid: partition p =
  destination node slot, columns = its incoming edges (self-loop first),
  padded slots masked to zero.
- Gathers of source-node features use per-column [128,1] indirect DMAs
  from a full (replicated) node-feature table in each core's HBM.
- Segment softmax + weighted aggregation are pure free-axis DVE/ACT ops
  (exp/leaky-relu on ACT, reductions on DVE). No PE in the edge phase.
- Dense projections run on the PE (x^T tiles as stationary weights).
- Layer-1 -> layer-2 table exchange is a host-side concat between two
  NEFF executions (collectives are unavailable in this runtime).
"""
import numpy as np
import ml_dtypes

N = 100000
E = 1600000
IN = 128
HID = 8
HEADS = 8
OUTC = 40
SLOPE = 0.2
NCORES = 8
P = 128

_CACHE = {}


# --------------------------------------------------------------------------
# host-side index preprocessing (pure integer/layout work)
# --------------------------------------------------------------------------

def _host_prep(edge_index, edge_weight):
    src = np.asarray(edge_index[0], dtype=np.int64)
    dst = np.asarray(edge_index[1], dtype=np.int64)
    ew = np.asarray(edge_weight, dtype=np.float32)
    n = N

    deg = np.bincount(dst, minlength=n).astype(np.int64)          # real in-degree
    order = np.argsort(-deg, kind="stable")                        # degree desc
    core_of = np.empty(n, dtype=np.int64)
    slot_of = np.empty(n, dtype=np.int64)
    core_of[order] = np.arange(n) % NCORES
    slot_of[order] = np.arange(n) // NCORES                        # rank within core

    n_per_core = [int((core_of == k).sum()) for k in range(NCORES)]
    nmax = max(n_per_core)
    G = (nmax + P - 1) // P
    NV = G * P                                                     # slots per core

    # vperm[k][v] = real node at slot v of core k  (-1 = pad)
    vperm = np.full((NCORES, NV), -1, dtype=np.int64)
    vperm[core_of, slot_of] = np.arange(n)

    # T1/T2 row of a real node: core*NV + slot
    rowof = core_of * NV + slot_of                                 # [n]

    # group schedule: NCH per group = max (deg+1) among its 128 nodes, over cores
    degv = np.zeros((NCORES, NV), dtype=np.int64)
    degv[core_of, slot_of] = deg + 1                               # incl self-loop
    nch_per = degv.reshape(NCORES, G, P).max(axis=2)               # [NCORES, G]
    NCHS = nch_per.max(axis=0).astype(np.int64)                    # [G]
    NCHS = np.maximum(NCHS, 1)
    CSTART = np.concatenate([[0], np.cumsum(NCHS)])                # [G+1]
    TOT = int(CSTART[-1])

    # place edges: edge e of dst d goes to (core_of[d], partition slot%P,
    # group slot//P, column 1 + rank-within-node); self-loop at column 0.
    e_core = core_of[dst]
    e_slot = slot_of[dst]
    ordr = np.argsort(e_core * NV + e_slot, kind="stable")
    sc, ss = e_core[ordr], e_slot[ordr]
    key = sc * NV + ss
    first = np.r_[True, key[1:] != key[:-1]]
    start_pos = np.flatnonzero(first)
    runlen = np.arange(len(key)) - np.repeat(start_pos, np.diff(np.r_[start_pos, len(key)]))
    e_col = np.empty(E, dtype=np.int64)
    e_col[ordr] = runlen + 1                                       # col 0 = self loop

    # metadata arrays [NCORES, P, TOT]
    SRC = np.zeros((NCORES, P, TOT), dtype=np.int32)
    M1 = np.zeros((NCORES, P, TOT), dtype=np.float32)
    M2 = np.zeros((NCORES, P, TOT), dtype=np.float32)

    e_g = e_slot // P
    e_p = e_slot % P
    e_c = CSTART[e_g] + e_col
    SRC[e_core, e_p, e_c] = rowof[src].astype(np.int32)
    M1[e_core, e_p, e_c] = 1.0
    M2[e_core, e_p, e_c] = ew

    # self loops (only for real nodes)
    rn = np.arange(n)
    s_g = slot_of // P
    SRC[core_of, slot_of % P, CSTART[s_g]] = rowof[rn].astype(np.int32)
    M1[core_of, slot_of % P, CSTART[s_g]] = 1.0
    M2[core_of, slot_of % P, CSTART[s_g]] = 1.0

    # self-row index per (core, partition, group): core*NV + g*128 + p
    SELF = (np.arange(NCORES)[:, None, None] * NV
            + np.arange(G)[None, None, :] * P
            + np.arange(P)[None, :, None]).astype(np.int32)       # [NCORES, P, G]

    return dict(G=G, NV=NV, NVG=NCORES * NV, NCHS=NCHS.tolist(),
                CSTART=CSTART.tolist(), TOT=TOT,
                vperm=vperm, SRC=SRC, M1=M1, M2=M2, SELF=SELF)


def _stage_x(x, vperm, NVG):
    """xvT [128, NVG] bf16: column (global slot) = x[real node]; pads 0."""
    xvT = np.zeros((IN, NVG), dtype=np.float32)
    flat = vperm.reshape(-1)
    valid = flat >= 0
    xvT[:, np.flatnonzero(valid)] = x[flat[valid]].T
    return xvT


def _blockdiag(att):
    """att [H, C] -> [H*C, H] block diagonal."""
    h, c = att.shape
    A = np.zeros((h * c, h), dtype=np.float32)
    for i in range(h):
        A[i * c:(i + 1) * c, i] = att[i]
    return A


# --------------------------------------------------------------------------
# device program builders
# --------------------------------------------------------------------------

def _build_neff1(meta):
    import concourse.bacc as bacc
    import concourse.mybir as mybir
    import concourse.tile as tile
    import concourse.bass as bass
    from concourse.masks import make_identity

    G, NV, NVG, TOT = meta["G"], meta["NV"], meta["NVG"], meta["TOT"]
    NCHS, CSTART = meta["NCHS"], meta["CSTART"]
    bf16, f32, i32 = mybir.dt.bfloat16, mybir.dt.float32, mybir.dt.int32
    AP = bass.AP

    nc = bacc.Bacc(num_devices=NCORES)
    t_xvT = nc.dram_tensor("xvT", [IN, NVG], f32, kind="ExternalInput")
    t_W1 = nc.dram_tensor("W1", [IN, HEADS * HID], f32, kind="ExternalInput")
    t_A1 = nc.dram_tensor("A1", [HEADS * HID, 2 * HEADS], f32, kind="ExternalInput")
    t_b1 = nc.dram_tensor("b1", [1, HEADS * HID], f32, kind="ExternalInput")
    t_W2 = nc.dram_tensor("W2", [HEADS * HID, OUTC], f32, kind="ExternalInput")
    t_A2 = nc.dram_tensor("A2", [OUTC, 2], f32, kind="ExternalInput")
    t_SRC = nc.dram_tensor("SRC", [P, TOT], i32, kind="ExternalInput")
    t_M1 = nc.dram_tensor("M1", [P, TOT], f32, kind="ExternalInput")
    t_M2 = nc.dram_tensor("M2", [P, TOT], f32, kind="ExternalInput")
    t_SELF = nc.dram_tensor("SELF", [P, G], i32, kind="ExternalInput")

    t_T1 = nc.dram_tensor("T1", [NVG, 80], f32)                   # [h(64)|asrc(8)|adst(8)]
    t_O1T = nc.dram_tensor("O1T", [HEADS * HID, NV], f32)         # elu(h1)^T local
    t_AG = nc.dram_tensor("AGIN", [NV, 42], f32, kind="ExternalOutput")

    with tile.TileContext(nc) as tc:
        with tc.tile_pool(name="const", bufs=1) as cpool:

            ident = cpool.tile([P, P], f32)
            make_identity(nc, ident[:])

            w1ee = cpool.tile([IN, 80], f32)
            w2e = cpool.tile([HEADS * HID, 42], f32)
            with tc.tile_pool(name="psetup", bufs=1, space="PSUM") as spp:
                # ---- build W1ee [128, 80] bf16 = [W1 | W1@A1] ----
                w1 = cpool.tile([IN, HEADS * HID], f32)
                nc.sync.dma_start(w1[:], t_W1[:])
                a1 = cpool.tile([HEADS * HID, 2 * HEADS], f32)
                nc.sync.dma_start(a1[:], t_A1[:])
                ps_w1t = spp.tile([HEADS * HID, IN], f32, space="PSUM")
                nc.tensor.transpose(out=ps_w1t[:], in_=w1[:], identity=ident[:])
                w1t = cpool.tile([HEADS * HID, IN], f32)
                nc.vector.tensor_copy(w1t[:], ps_w1t[:])
                ps_w1a = spp.tile([IN, 2 * HEADS], f32, space="PSUM")
                nc.tensor.matmul(out=ps_w1a[:], lhsT=w1t[:], rhs=a1[:],
                                 start=True, stop=True)
                nc.vector.tensor_copy(w1ee[:, 0:64], w1[:])
                nc.vector.tensor_copy(w1ee[:, 64:80], ps_w1a[:])

                # ---- build W2e [64, 42] bf16 = [W2 | W2@A2] ----
                w2 = cpool.tile([HEADS * HID, OUTC], f32)
                nc.sync.dma_start(w2[:], t_W2[:])
                a2 = cpool.tile([OUTC, 2], f32)
                nc.sync.dma_start(a2[:], t_A2[:])
                ps_w2t = spp.tile([OUTC, HEADS * HID], f32, space="PSUM")
                nc.tensor.transpose(out=ps_w2t[:], in_=w2[:],
                                    identity=ident[0:HEADS * HID, 0:HEADS * HID])
                w2t = cpool.tile([OUTC, HEADS * HID], f32)
                nc.vector.tensor_copy(w2t[:], ps_w2t[:])
                ps_w2a = spp.tile([HEADS * HID, 2], f32, space="PSUM")
                nc.tensor.matmul(out=ps_w2a[:], lhsT=w2t[:], rhs=a2[:],
                                 start=True, stop=True)
                nc.vector.tensor_copy(w2e[:, 0:OUTC], w2[:])
                nc.vector.tensor_copy(w2e[:, OUTC:42], ps_w2a[:])

            # ---- bias1 replicated [128, 64] ----
            b1mat = cpool.tile([P, HEADS * HID], f32)
            nc.sync.dma_start(b1mat[:], AP(t_b1, 0, [[0, P], [1, HEADS * HID]]))

            # ---- metadata resident in SBUF ----
            m_src = cpool.tile([P, TOT], i32)
            nc.sync.dma_start(m_src[:], t_SRC[:])
            m_m1 = cpool.tile([P, TOT], f32)
            nc.sync.dma_start(m_m1[:], t_M1[:])
            m_m2 = cpool.tile([P, TOT], f32)
            nc.sync.dma_start(m_m2[:], t_M2[:])
            m_self = cpool.tile([P, G], i32)
            nc.sync.dma_start(m_self[:], t_SELF[:])

            from contextlib import ExitStack
            _ps = ExitStack()
            pool = _ps.enter_context(tc.tile_pool(name="work", bufs=3))
            ppool = _ps.enter_context(tc.tile_pool(name="psum", bufs=2, space="PSUM"))

            # ---- L1 projection: T1 rows = [x@W1 | asrc | adst] ----
            XW = 1024
            for t0 in range(0, NVG, XW):
                tw = min(XW, NVG - t0)
                xw = pool.tile([IN, XW], f32, tag="xw")
                nc.sync.dma_start(xw[:, 0:tw], t_xvT[:, t0:t0 + tw])
                for j in range(0, tw, P):
                    ps = ppool.tile([P, 80], f32, space="PSUM", tag="pj")
                    nc.tensor.matmul(out=ps[:], lhsT=xw[:, j:j + P],
                                     rhs=w1ee[:], start=True, stop=True)
                    ot = pool.tile([P, 80], f32, tag="ot")
                    nc.vector.tensor_copy(ot[:], ps[:])
                    nc.sync.dma_start(t_T1[t0 + j:t0 + j + P, :], ot[:])

            # ---- L1 edge phase ----
            H = HEADS
            for g in range(G):
                NCH = NCHS[g]
                cs = CSTART[g]
                gb = pool.tile([P, NCH, 80], f32, tag="gb")
                for c in range(NCH):
                    nc.gpsimd.indirect_dma_start(
                        out=gb[:, c, :], out_offset=None, in_=t_T1[:],
                        in_offset=bass.IndirectOffsetOnAxis(
                            ap=m_src[:, cs + c:cs + c + 1], axis=0))
                ad = pool.tile([P, H], f32, tag="ad")
                nc.gpsimd.indirect_dma_start(
                    out=ad[:], out_offset=None, in_=t_T1[:],
                    in_offset=bass.IndirectOffsetOnAxis(ap=m_self[:, g:g + 1], axis=0),
                    element_offset=72)

                gbo = gb[:].offset
                pstep = gb[:].ap[0][0]
                # l = asrc(by src) + adst(own), h-major [128, H, NCH]
                asrc_v = AP(gb.tensor, gbo + 64, [[pstep, P], [1, H], [80, NCH]])
                ad_v = AP(ad.tensor, ad[:].offset, [[ad[:].ap[0][0], P], [1, H], [0, NCH]])
                lg = pool.tile([P, H, NCH], f32, tag="lg")
                nc.vector.tensor_tensor(lg[:], asrc_v, ad_v, mybir.AluOpType.add)
                lr = pool.tile([P, H, NCH], f32, tag="lr")
                nc.vector.scalar_tensor_tensor(
                    out=lr[:], in0=lg[:], scalar=SLOPE, in1=lg[:],
                    op0=mybir.AluOpType.mult, op1=mybir.AluOpType.max)
                ex = pool.tile([P, H, NCH], f32, tag="ex")
                nc.scalar.activation(ex[:], lr[:], mybir.ActivationFunctionType.Exp)

                m1_v = AP(m_m1.tensor, m_m1[:].offset + cs,
                          [[m_m1[:].ap[0][0], P], [0, H], [1, NCH]])
                m2_v = AP(m_m2.tensor, m_m2[:].offset + cs,
                          [[m_m2[:].ap[0][0], P], [0, H], [1, NCH]])
                exd = pool.tile([P, H, NCH], f32, tag="exd")
                nc.vector.tensor_tensor(exd[:], ex[:], m1_v, mybir.AluOpType.mult)
                exm = pool.tile([P, H, NCH], f32, tag="exm")
                nc.vector.tensor_tensor(exm[:], ex[:], m2_v, mybir.AluOpType.mult)

                den = pool.tile([P, H], f32, tag="den")
                nc.vector.tensor_reduce(den[:], exd[:], mybir.AxisListType.X,
                                        mybir.AluOpType.add)
                den2 = pool.tile([P, H], f32, tag="den2")
                nc.vector.tensor_scalar_add(den2[:], den[:], 1e-16)
                rd = pool.tile([P, H], f32, tag="rd")
                nc.vector.reciprocal(rd[:], den2[:])

                # msgw[p, h, j, c] = h_src[p, c, 8h+j] * exm[p, h, c]
                gh_v = AP(gb.tensor, gbo, [[pstep, P], [8, H], [1, HID], [80, NCH]])
                exm_o = exm[:].offset
                exm_p = exm[:].ap[0][0]
                exm_v = AP(exm.tensor, exm_o, [[exm_p, P], [NCH, H], [0, HID], [1, NCH]])
                msgw = pool.tile([P, H, HID, NCH], f32, tag="msgw")
                nc.vector.tensor_tensor(msgw[:], gh_v, exm_v, mybir.AluOpType.mult)
                U = pool.tile([P, H, HID], f32, tag="U")
                nc.vector.tensor_reduce(U[:], msgw[:], mybir.AxisListType.X,
                                        mybir.AluOpType.add)

                rd_v = AP(rd.tensor, rd[:].offset, [[rd[:].ap[0][0], P], [1, H], [0, HID]])
                t2 = pool.tile([P, H * HID], f32, tag="t2")
                nc.vector.tensor_tensor(
                    t2[:].rearrange("p (a b) -> p a b", a=H), U[:], rd_v,
                    mybir.AluOpType.mult)
                t3 = pool.tile([P, H * HID], f32, tag="t3")
                nc.vector.tensor_add(t3[:], t2[:], b1mat[:])
                # elu
                neg = pool.tile([P, H * HID], f32, tag="neg")
                nc.vector.tensor_scalar_min(neg[:], t3[:], 0.0)
                een = pool.tile([P, H * HID], f32, tag="een")
                nc.scalar.activation(een[:], neg[:], mybir.ActivationFunctionType.Exp)
                pos = pool.tile([P, H * HID], f32, tag="pos")
                nc.vector.tensor_scalar_max(pos[:], t3[:], 0.0)
                h1 = pool.tile([P, H * HID], f32, tag="h1")
                nc.vector.scalar_tensor_tensor(
                    out=h1[:], in0=een[:], scalar=-1.0, in1=pos[:],
                    op0=mybir.AluOpType.add, op1=mybir.AluOpType.add)
                # transpose -> O1T[:, g*128:(g+1)*128]
                ps_tr = ppool.tile([H * HID, P], f32, space="PSUM", tag="ptr")
                nc.tensor.transpose(out=ps_tr[:], in_=h1[:], identity=ident[:])
                o1 = pool.tile([H * HID, P], f32, tag="o1")
                nc.vector.tensor_copy(o1[:], ps_tr[:])
                nc.sync.dma_start(t_O1T[:, g * P:(g + 1) * P], o1[:])

            # ---- L2 projection: AGIN rows = [h2' | asrc2 | adst2] ----
            for g in range(G):
                lh = pool.tile([H * HID, P], f32, tag="lh")
                nc.sync.dma_start(lh[:], t_O1T[:, g * P:(g + 1) * P])
                ps2 = ppool.tile([P, 42], f32, space="PSUM", tag="p2")
                nc.tensor.matmul(out=ps2[:], lhsT=lh[:], rhs=w2e[:], start=True, stop=True)
                og = pool.tile([P, 42], f32, tag="og")
                nc.vector.tensor_copy(og[:], ps2[:])
                nc.sync.dma_start(t_AG[g * P:(g + 1) * P, :], og[:])

            _ps.close()

    nc.finalize()
    return nc


def _build_neff2(meta):
    import concourse.bacc as bacc
    import concourse.mybir as mybir
    import concourse.tile as tile
    import concourse.bass as bass

    G, NV, NVG, TOT = meta["G"], meta["NV"], meta["NVG"], meta["TOT"]
    NCHS, CSTART = meta["NCHS"], meta["CSTART"]
    bf16, f32, i32 = mybir.dt.bfloat16, mybir.dt.float32, mybir.dt.int32
    AP = bass.AP

    nc = bacc.Bacc(num_devices=NCORES)
    t_T2 = nc.dram_tensor("T2", [NVG, 42], f32, kind="ExternalInput")
    t_b2 = nc.dram_tensor("b2", [1, OUTC], f32, kind="ExternalInput")
    t_SRC = nc.dram_tensor("SRC", [P, TOT], i32, kind="ExternalInput")
    t_M1 = nc.dram_tensor("M1", [P, TOT], f32, kind="ExternalInput")
    t_M2 = nc.dram_tensor("M2", [P, TOT], f32, kind="ExternalInput")
    t_SELF = nc.dram_tensor("SELF", [P, G], i32, kind="ExternalInput")
    t_OUT = nc.dram_tensor("OUT2", [NV, OUTC], f32, kind="ExternalOutput")

    with tile.TileContext(nc) as tc:
        with tc.tile_pool(name="const", bufs=1) as cpool, \
             tc.tile_pool(name="work", bufs=3) as pool:

            b2mat = cpool.tile([P, OUTC], f32)
            nc.sync.dma_start(b2mat[:], AP(t_b2, 0, [[0, P], [1, OUTC]]))
            m_src = cpool.tile([P, TOT], i32)
            nc.sync.dma_start(m_src[:], t_SRC[:])
            m_m1 = cpool.tile([P, TOT], f32)
            nc.sync.dma_start(m_m1[:], t_M1[:])
            m_m2 = cpool.tile([P, TOT], f32)
            nc.sync.dma_start(m_m2[:], t_M2[:])
            m_self = cpool.tile([P, G], i32)
            nc.sync.dma_start(m_self[:], t_SELF[:])

            for g in range(G):
                NCH = NCHS[g]
                cs = CSTART[g]
                gb = pool.tile([P, NCH, 42], f32, tag="gb")
                for c in range(NCH):
                    nc.gpsimd.indirect_dma_start(
                        out=gb[:, c, :], out_offset=None, in_=t_T2[:],
                        in_offset=bass.IndirectOffsetOnAxis(
                            ap=m_src[:, cs + c:cs + c + 1], axis=0))
                ad = pool.tile([P, 1], f32, tag="ad")
                nc.gpsimd.indirect_dma_start(
                    out=ad[:], out_offset=None, in_=t_T2[:],
                    in_offset=bass.IndirectOffsetOnAxis(ap=m_self[:, g:g + 1], axis=0),
                    element_offset=41)

                gbo = gb[:].offset
                pstep = gb[:].ap[0][0]
                asrc_v = AP(gb.tensor, gbo + 40, [[pstep, P], [1, 1], [42, NCH]])
                ad_v = AP(ad.tensor, ad[:].offset, [[ad[:].ap[0][0], P], [1, 1], [0, NCH]])
                lg = pool.tile([P, 1, NCH], f32, tag="lg")
                nc.vector.tensor_tensor(lg[:], asrc_v, ad_v, mybir.AluOpType.add)
                lr = pool.tile([P, 1, NCH], f32, tag="lr")
                nc.vector.scalar_tensor_tensor(
                    out=lr[:], in0=lg[:], scalar=SLOPE, in1=lg[:],
                    op0=mybir.AluOpType.mult, op1=mybir.AluOpType.max)
                ex = pool.tile([P, 1, NCH], f32, tag="ex")
                nc.scalar.activation(ex[:], lr[:], mybir.ActivationFunctionType.Exp)

                m1_v = AP(m_m1.tensor, m_m1[:].offset + cs,
                          [[m_m1[:].ap[0][0], P], [0, 1], [1, NCH]])
                m2_v = AP(m_m2.tensor, m_m2[:].offset + cs,
                          [[m_m2[:].ap[0][0], P], [0, 1], [1, NCH]])
                exd = pool.tile([P, 1, NCH], f32, tag="exd")
                nc.vector.tensor_tensor(exd[:], ex[:], m1_v, mybir.AluOpType.mult)
                exm = pool.tile([P, 1, NCH], f32, tag="exm")
                nc.vector.tensor_tensor(exm[:], ex[:], m2_v, mybir.AluOpType.mult)

                den = pool.tile([P, 1], f32, tag="den")
                nc.vector.tensor_reduce(den[:], exd[:], mybir.AxisListType.X,
                                        mybir.AluOpType.add)
                den2 = pool.tile([P, 1], f32, tag="den2")
                nc.vector.tensor_scalar_add(den2[:], den[:], 1e-16)
                rd = pool.tile([P, 1], f32, tag="rd")
                nc.vector.reciprocal(rd[:], den2[:])

                gh_v = AP(gb.tensor, gbo, [[pstep, P], [1, OUTC], [42, NCH]])
                exm_v = AP(exm.tensor, exm[:].offset,
                           [[exm[:].ap[0][0], P], [0, OUTC], [1, NCH]])
                msgw = pool.tile([P, OUTC, NCH], f32, tag="msgw")
                nc.vector.tensor_tensor(msgw[:], gh_v, exm_v, mybir.AluOpType.mult)
                U = pool.tile([P, OUTC], f32, tag="U")
                nc.vector.tensor_reduce(U[:], msgw[:], mybir.AxisListType.X,
                                        mybir.AluOpType.add)

                rd_v = AP(rd.tensor, rd[:].offset, [[rd[:].ap[0][0], P], [0, OUTC]])
                t2 = pool.tile([P, OUTC], f32, tag="t2")
                nc.vector.tensor_tensor(t2[:], U[:], rd_v, mybir.AluOpType.mult)
                t3 = pool.tile([P, OUTC], f32, tag="t3")
                nc.vector.tensor_add(t3[:], t2[:], b2mat[:])
                nc.sync.dma_start(t_OUT[g * P:(g + 1) * P, :], t3[:])

    nc.finalize()
    return nc


# --------------------------------------------------------------------------
# entry point
# --------------------------------------------------------------------------

def kernel(x, edge_index, edge_weight, W1, att_src1, att_dst1, bias1,
           W2, att_src2, att_dst2, bias2):
    SpmdRunner = _inline_runner()

    x = np.asarray(x, dtype=np.float32)
    W1 = np.asarray(W1, dtype=np.float32)
    W2 = np.asarray(W2, dtype=np.float32)
    bias1 = np.asarray(bias1, dtype=np.float32)
    bias2 = np.asarray(bias2, dtype=np.float32)

    import hashlib
    h = hashlib.sha1()
    h.update(np.ascontiguousarray(edge_index).tobytes())
    h.update(np.ascontiguousarray(edge_weight).tobytes())
    key = h.hexdigest()
    if _CACHE.get("key") != key:
        _CACHE.clear()
        _CACHE["key"] = key
        _CACHE["meta"] = _host_prep(edge_index, edge_weight)
    meta = _CACHE["meta"]

    xvT = _stage_x(x, meta["vperm"], meta["NVG"])
    A1 = np.concatenate(
        [_blockdiag(np.asarray(att_src1, np.float32)),
         _blockdiag(np.asarray(att_dst1, np.float32))], axis=1)    # [64, 16]
    A2 = np.concatenate(
        [np.asarray(att_src2, np.float32).reshape(OUTC, 1),
         np.asarray(att_dst2, np.float32).reshape(OUTC, 1)], axis=1)  # [40, 2]

    if "nc1" not in _CACHE:
        _CACHE["nc1"] = _build_neff1(meta)
        _CACHE["run1"] = SpmdRunner(_CACHE["nc1"], NCORES)
    run1 = _CACHE["run1"]

    in_maps1 = []
    for k in range(NCORES):
        in_maps1.append({
            "xvT": xvT, "W1": W1, "A1": A1, "b1": bias1.reshape(1, -1),
            "W2": W2, "A2": A2,
            "SRC": meta["SRC"][k], "M1": np.asarray(meta["M1"][k]),
            "M2": np.asarray(meta["M2"][k]), "SELF": meta["SELF"][k],
        })
    args1 = run1.prepare(in_maps1)
    _CACHE["args1_cached"] = args1
    res1 = run1.results(run1.run(args1))

    T2full = np.concatenate([r["AGIN"] for r in res1], axis=0)     # [NVG, 42] bf16

    if "nc2" not in _CACHE:
        _CACHE["nc2"] = _build_neff2(meta)
        _CACHE["run2"] = SpmdRunner(_CACHE["nc2"], NCORES)
    run2 = _CACHE["run2"]

    in_maps2 = []
    for k in range(NCORES):
        in_maps2.append({
            "T2": T2full, "b2": bias2.reshape(1, -1),
            "SRC": meta["SRC"][k], "M1": np.asarray(meta["M1"][k]),
            "M2": np.asarray(meta["M2"][k]), "SELF": meta["SELF"][k],
        })
    args2 = run2.prepare(in_maps2)
    _CACHE["args2_cached"] = args2
    res2 = run2.results(run2.run(args2))

    out = np.zeros((N, OUTC), dtype=np.float32)
    for k in range(NCORES):
        vp = meta["vperm"][k]
        valid = vp >= 0
        out[vp[valid]] = res2[k]["OUT2"][np.flatnonzero(valid)]
    return out


def _inline_runner():
    """Self-contained copy of runner.SpmdRunner for harness environments."""
    import time
    import jax
    from jax.sharding import Mesh, PartitionSpec
    from jax.experimental.shard_map import shard_map
    import concourse.mybir as mybir
    from concourse import bass2jax
    from concourse.bass2jax import _bass_exec_p, partition_id_tensor

    class SpmdRunner:
        def __init__(self, nc, n_cores):
            bass2jax.install_neuronx_cc_hook()
            self.nc = nc
            self.n_cores = n_cores
            in_names, out_names, out_avals, zero_outs = [], [], [], []
            partition_name = (nc.partition_id_tensor.name
                              if nc.partition_id_tensor else None)
            for alloc in nc.m.functions[0].allocations:
                if not isinstance(alloc, mybir.MemoryLocationSet):
                    continue
                name = alloc.memorylocations[0].name
                if alloc.kind == "ExternalInput":
                    if name != partition_name:
                        in_names.append(name)
                elif alloc.kind == "ExternalOutput":
                    shape = tuple(alloc.tensor_shape)
                    dtype = mybir.dt.np(alloc.dtype)
                    out_names.append(name)
                    out_avals.append(jax.core.ShapedArray(shape, dtype))
                    zero_outs.append(np.zeros(shape, dtype))
            self.in_names = list(in_names)
            self.out_names, self.out_avals, self.zero_outs = out_names, out_avals, zero_outs
            n_params, n_outs = len(in_names), len(out_avals)
            all_in = in_names + out_names + ([partition_name] if partition_name else [])

            def _body(*args):
                operands = list(args)
                if partition_name is not None:
                    operands.append(partition_id_tensor())
                return tuple(_bass_exec_p.bind(
                    *operands, out_avals=tuple(out_avals), in_names=tuple(all_in),
                    out_names=tuple(out_names), lowering_input_output_aliases=(),
                    sim_require_finite=True, sim_require_nnan=True, nc=nc))

            devices = jax.devices()[:n_cores]
            mesh = Mesh(np.asarray(devices), ("core",))
            in_specs = (PartitionSpec("core"),) * (n_params + n_outs)
            out_specs = (PartitionSpec("core"),) * n_outs
            self.fn = jax.jit(shard_map(_body, mesh=mesh, in_specs=in_specs,
                                        out_specs=out_specs, check_rep=False),
                              keep_unused=True)
            self.n_params, self.n_outs = n_params, n_outs
            self._mesh = mesh

        def prepare(self, in_maps, device_put=True):
            import jax
            from jax.sharding import PartitionSpec
            per_core = [[np.asarray(m[nm]) for nm in self.in_names] for m in in_maps]
            args = [np.concatenate([per_core[c][i] for c in range(self.n_cores)], axis=0)
                    for i in range(self.n_params)]
            args += [np.zeros((self.n_cores * z.shape[0], *z.shape[1:]), z.dtype)
                     for z in self.zero_outs]
            if device_put:
                sh = jax.sharding.NamedSharding(self._mesh, PartitionSpec("core"))
                args = [jax.device_put(a, sh) for a in args]
                jax.block_until_ready(args)
            return args

        def run(self, args):
            import jax
            outs = self.fn(*args)
            jax.block_until_ready(outs)
            return outs

        def results(self, outs):
            return [{nm: np.asarray(outs[i]).reshape(
                        self.n_cores, *self.out_avals[i].shape)[c]
                     for i, nm in enumerate(self.out_names)}
                    for c in range(self.n_cores)]

    return SpmdRunner



# revision 17
# speedup vs baseline: 1.0398x; 1.0398x over previous
"""2-layer GAT on 8 Trainium2 NeuronCores (Bass/Tile).

Strategy (edge-parallel, per the sharding hint: "each device holds a shard
of edge_index/edge_weight plus replicated (or gathered halo) node features"):

- Destination nodes are assigned round-robin by degree rank to the 8 cores;
  per core, dst slots are grouped 128 per partition-group. A group's edges
  live in a [128 partitions = dst slot, NCH columns] grid (self-loop first,
  padded slots masked to zero).
- The host stages each core's edge shard WITH ITS GATHERED HALO: for every
  grid slot, the raw source-node feature vector (bf16) — i.e. x[src] laid
  out in grid order. No indirect/gather DMA is ever issued on device (the
  SWDGE gather path costs ~16us/instruction on this runtime); the device
  streams the halo grid sequentially at full HBM bandwidth.
- ALL math runs on device: L1 projection of every (edge-slot) source vector
  on the PE (x@[W1|W1@A1] per grid column), segment softmax + weighted
  aggregation on DVE/ACT, ELU, L2 projection on PE.
- Between the two layer NEFFs the host re-shards the layer-1 activations
  the same way (gathered halo per edge slot) — replacing the cross-device
  exchange; segment reductions stay core-local because edges are
  dst-partitioned.
"""
import numpy as np
import ml_dtypes

N = 100000
E = 1600000
IN = 128
HID = 8
HEADS = 8
OUTC = 40
SLOPE = 0.2
NCORES = 8
P = 128
PCOLS = 6        # grid columns projected per PSUM tile (6*80 = 480 <= 512)

BF16 = ml_dtypes.bfloat16

_CACHE = {}


# --------------------------------------------------------------------------
# host-side index preprocessing (pure integer/layout work)
# --------------------------------------------------------------------------

def _host_prep(edge_index, edge_weight):
    src = np.asarray(edge_index[0], dtype=np.int64)
    dst = np.asarray(edge_index[1], dtype=np.int64)
    ew = np.asarray(edge_weight, dtype=np.float32)
    n = N

    deg = np.bincount(dst, minlength=n).astype(np.int64)          # real in-degree
    order = np.argsort(-deg, kind="stable")                        # degree desc
    core_of = np.empty(n, dtype=np.int64)
    slot_of = np.empty(n, dtype=np.int64)
    core_of[order] = np.arange(n) % NCORES
    slot_of[order] = np.arange(n) // NCORES                        # rank within core

    n_per_core = [int((core_of == k).sum()) for k in range(NCORES)]
    nmax = max(n_per_core)
    G = (nmax + P - 1) // P
    NV = G * P                                                     # slots per core

    # vperm[k][v] = real node at slot v of core k  (-1 = pad)
    vperm = np.full((NCORES, NV), -1, dtype=np.int64)
    vperm[core_of, slot_of] = np.arange(n)

    rowof = core_of * NV + slot_of                                 # [n] T2 row

    # group schedule: NCH per group = max (deg+1) among its 128 nodes, over cores
    degv = np.zeros((NCORES, NV), dtype=np.int64)
    degv[core_of, slot_of] = deg + 1                               # incl self-loop
    nch_per = degv.reshape(NCORES, G, P).max(axis=2)               # [NCORES, G]
    NCHS = nch_per.max(axis=0).astype(np.int64)                    # [G]
    NCHS = np.maximum(NCHS, 1)
    CSTART = np.concatenate([[0], np.cumsum(NCHS)])                # [G+1]
    TOT = int(CSTART[-1])

    # place edges: edge e of dst d -> (core_of[d], partition slot%P,
    # group slot//P, column 1 + rank-within-node); self-loop at column 0.
    e_core = core_of[dst]
    e_slot = slot_of[dst]
    ordr = np.argsort(e_core * NV + e_slot, kind="stable")
    sc, ss = e_core[ordr], e_slot[ordr]
    key = sc * NV + ss
    first = np.r_[True, key[1:] != key[:-1]]
    start_pos = np.flatnonzero(first)
    runlen = np.arange(len(key)) - np.repeat(start_pos, np.diff(np.r_[start_pos, len(key)]))
    e_col = np.empty(E, dtype=np.int64)
    e_col[ordr] = runlen + 1                                       # col 0 = self loop

    # per-core grid metadata
    SRCN = np.zeros((NCORES, TOT, P), dtype=np.int64)              # source node id
    M2 = np.zeros((NCORES, P, TOT), dtype=np.float32)              # ew * validity
    M1 = np.zeros((NCORES, P, TOT), dtype=np.float32)              # validity

    e_g = e_slot // P
    e_p = e_slot % P
    e_c = CSTART[e_g] + e_col
    SRCN[e_core, e_c, e_p] = src
    M2[e_core, e_p, e_c] = ew
    M1[e_core, e_p, e_c] = 1.0

    # self loops (only for real nodes)
    rn = np.arange(n)
    s_c = CSTART[slot_of // P]
    SRCN[core_of, s_c, slot_of % P] = rn
    M2[core_of, slot_of % P, s_c] = 1.0
    M1[core_of, slot_of % P, s_c] = 1.0

    # L2 grid source rows: rowof[SRCN]; pads point at row 0
    SRCROW = rowof[SRCN]                                           # [NCORES, TOT, P]

    return dict(G=G, NV=NV, NVG=NCORES * NV, NCHS=NCHS.tolist(),
                CSTART=CSTART.tolist(), TOT=TOT,
                m1_same=bool(np.array_equal(M1, M2)),
                vperm=vperm, SRCN=SRCN, SRCROW=SRCROW, M1=M1, M2=M2)


def _blockdiag(att):
    """att [H, C] -> [H*C, H] block diagonal."""
    h, c = att.shape
    A = np.zeros((h * c, h), dtype=np.float32)
    for i in range(h):
        A[i * c:(i + 1) * c, i] = att[i]
    return A


# --------------------------------------------------------------------------
# device program builders
# --------------------------------------------------------------------------

def _build_neff1(meta):
    import concourse.bacc as bacc
    import concourse.mybir as mybir
    import concourse.tile as tile
    import concourse.bass as bass
    from concourse.masks import make_identity

    G, NV, TOT = meta["G"], meta["NV"], meta["TOT"]
    NCHS, CSTART = meta["NCHS"], meta["CSTART"]
    m1_same = meta["m1_same"]
    bf16, f32 = mybir.dt.bfloat16, mybir.dt.float32
    AP = bass.AP
    H = HEADS

    nc = bacc.Bacc(num_devices=NCORES)
    t_xg = nc.dram_tensor("XG", [IN, TOT * P], bf16, kind="ExternalInput")
    t_xn = nc.dram_tensor("XN", [IN, NV], bf16, kind="ExternalInput")
    t_W1 = nc.dram_tensor("W1", [IN, HEADS * HID], f32, kind="ExternalInput")
    t_A1 = nc.dram_tensor("A1", [HEADS * HID, 2 * HEADS], f32, kind="ExternalInput")
    t_b1 = nc.dram_tensor("b1", [1, HEADS * HID], f32, kind="ExternalInput")
    t_W2 = nc.dram_tensor("W2", [HEADS * HID, OUTC], f32, kind="ExternalInput")
    t_A2 = nc.dram_tensor("A2", [OUTC, 2], f32, kind="ExternalInput")
    if not m1_same:
        t_M1 = nc.dram_tensor("M1", [P, TOT], f32, kind="ExternalInput")
    t_M2 = nc.dram_tensor("M2", [P, TOT], f32, kind="ExternalInput")

    t_O1T = nc.dram_tensor("O1T", [HEADS * HID, NV], f32)         # elu(h1)^T local
    t_AG = nc.dram_tensor("AGIN", [NV, 42], f32, kind="ExternalOutput")

    with tile.TileContext(nc) as tc:
        with tc.tile_pool(name="const", bufs=1) as cpool:

            ident = cpool.tile([P, P], f32)
            make_identity(nc, ident[:])

            w1eeb = cpool.tile([IN, 80], bf16)
            w1adb = cpool.tile([IN, H], bf16)
            w2e = cpool.tile([HEADS * HID, 42], f32)
            with tc.tile_pool(name="psetup", bufs=1, space="PSUM") as spp:
                # ---- build [W1 | W1@A1src] bf16 and W1@A1dst bf16 ----
                w1 = cpool.tile([IN, HEADS * HID], f32)
                nc.sync.dma_start(w1[:], t_W1[:])
                a1 = cpool.tile([HEADS * HID, 2 * HEADS], f32)
                nc.sync.dma_start(a1[:], t_A1[:])
                ps_w1t = spp.tile([HEADS * HID, IN], f32, space="PSUM")
                nc.tensor.transpose(out=ps_w1t[:], in_=w1[:], identity=ident[:])
                w1t = cpool.tile([HEADS * HID, IN], f32)
                nc.vector.tensor_copy(w1t[:], ps_w1t[:])
                ps_w1a = spp.tile([IN, 2 * HEADS], f32, space="PSUM")
                nc.tensor.matmul(out=ps_w1a[:], lhsT=w1t[:], rhs=a1[:],
                                 start=True, stop=True)
                nc.vector.tensor_copy(w1eeb[:, 0:64], w1[:])
                nc.vector.tensor_copy(w1eeb[:, 64:72], ps_w1a[:, 0:H])
                nc.vector.tensor_copy(w1eeb[:, 72:80], ps_w1a[:, H:2 * H])
                nc.vector.tensor_copy(w1adb[:], ps_w1a[:, H:2 * H])

                # ---- build W2e [64, 42] f32 = [W2 | W2@A2] ----
                w2 = cpool.tile([HEADS * HID, OUTC], f32)
                nc.sync.dma_start(w2[:], t_W2[:])
                a2 = cpool.tile([OUTC, 2], f32)
                nc.sync.dma_start(a2[:], t_A2[:])
                ps_w2t = spp.tile([OUTC, HEADS * HID], f32, space="PSUM")
                nc.tensor.transpose(out=ps_w2t[:], in_=w2[:],
                                    identity=ident[0:HEADS * HID, 0:HEADS * HID])
                w2t = cpool.tile([OUTC, HEADS * HID], f32)
                nc.vector.tensor_copy(w2t[:], ps_w2t[:])
                ps_w2a = spp.tile([HEADS * HID, 2], f32, space="PSUM")
                nc.tensor.matmul(out=ps_w2a[:], lhsT=w2t[:], rhs=a2[:],
                                 start=True, stop=True)
                nc.vector.tensor_copy(w2e[:, 0:OUTC], w2[:])
                nc.vector.tensor_copy(w2e[:, OUTC:42], ps_w2a[:])

            # ---- bias1 replicated [128, 64] ----
            b1mat = cpool.tile([P, HEADS * HID], f32)
            nc.sync.dma_start(b1mat[:], AP(t_b1, 0, [[0, P], [1, HEADS * HID]]))

            # ---- masks resident in SBUF ----
            if not m1_same:
                m_m1 = cpool.tile([P, TOT], f32)
                nc.sync.dma_start(m_m1[:], t_M1[:])
            m_m2 = cpool.tile([P, TOT], f32)
            nc.sync.dma_start(m_m2[:], t_M2[:])

            from contextlib import ExitStack
            _ps = ExitStack()
            pool = _ps.enter_context(tc.tile_pool(name="work", bufs=3))
            ppool = _ps.enter_context(tc.tile_pool(name="psum", bufs=2, space="PSUM"))

            # ---- per-slot own adst: project local node features ----
            adall = cpool.tile([P, G, H], f32)
            for g in range(G):
                xn = pool.tile([IN, P], bf16, tag="xn")
                nc.sync.dma_start(xn[:], t_xn[:, g * P:(g + 1) * P])
                ps_ad = ppool.tile([P, H], f32, space="PSUM", tag="pad")
                nc.tensor.matmul(out=ps_ad[:], lhsT=xn[:], rhs=w1adb[:],
                                 start=True, stop=True)
                nc.scalar.copy(adall[:, g, :], ps_ad[:])

            # ---- edge phase: per group project halo grid then segment ops ----
            for g in range(G):
                NCH = NCHS[g]
                cs = CSTART[g]
                xg = pool.tile([IN, NCH * P], bf16, tag="xg")
                nc.sync.dma_start(xg[:], t_xg[:, cs * P:(cs + NCH) * P])

                gb = pool.tile([P, NCH, 80], f32, tag="gb")
                gb_flat = gb[:].rearrange("p a b -> p (a b)")
                for c0 in range(0, NCH, PCOLS):
                    cw = min(PCOLS, NCH - c0)
                    ps = ppool.tile([P, PCOLS * 80], f32, space="PSUM", tag="pj")
                    for j in range(cw):
                        nc.tensor.matmul(
                            out=ps[:, j * 80:(j + 1) * 80],
                            lhsT=xg[:, (c0 + j) * P:(c0 + j + 1) * P],
                            rhs=w1eeb[:], start=True, stop=True)
                    nc.scalar.copy(gb_flat[:, c0 * 80:(c0 + cw) * 80],
                                   ps[:, 0:cw * 80])

                gbo = gb[:].offset
                pstep = gb[:].ap[0][0]
                # l = asrc(by src) + adst(own), h-major [128, H, NCH]
                asrc_v = AP(gb.tensor, gbo + 64, [[pstep, P], [1, H], [80, NCH]])
                adg = adall[:, g, :]
                ad_v = AP(adall.tensor, adg.offset, [[adg.ap[0][0], P], [1, H], [0, NCH]])
                lg = pool.tile([P, H, NCH], f32, tag="lg")
                nc.vector.tensor_tensor(lg[:], asrc_v, ad_v, mybir.AluOpType.add)
                lr = pool.tile([P, H, NCH], f32, tag="lr")
                nc.vector.scalar_tensor_tensor(
                    out=lr[:], in0=lg[:], scalar=SLOPE, in1=lg[:],
                    op0=mybir.AluOpType.mult, op1=mybir.AluOpType.max)
                ex = pool.tile([P, H, NCH], f32, tag="ex")
                nc.scalar.activation(ex[:], lr[:], mybir.ActivationFunctionType.Exp)

                m2_v = AP(m_m2.tensor, m_m2[:].offset + cs,
                          [[m_m2[:].ap[0][0], P], [0, H], [1, NCH]])
                exm = pool.tile([P, H, NCH], f32, tag="exm")
                nc.vector.tensor_tensor(exm[:], ex[:], m2_v, mybir.AluOpType.mult)
                if m1_same:
                    exd = exm
                else:
                    m1_v = AP(m_m1.tensor, m_m1[:].offset + cs,
                              [[m_m1[:].ap[0][0], P], [0, H], [1, NCH]])
                    exd = pool.tile([P, H, NCH], f32, tag="exd")
                    nc.vector.tensor_tensor(exd[:], ex[:], m1_v, mybir.AluOpType.mult)

                den = pool.tile([P, H], f32, tag="den")
                nc.vector.tensor_reduce(den[:], exd[:], mybir.AxisListType.X,
                                        mybir.AluOpType.add)
                den2 = pool.tile([P, H], f32, tag="den2")
                nc.vector.tensor_scalar_add(den2[:], den[:], 1e-16)
                rd = pool.tile([P, H], f32, tag="rd")
                nc.vector.reciprocal(rd[:], den2[:])

                # msgw[p, h, j, c] = h_src[p, c, 8h+j] * exm[p, h, c]
                gh_v = AP(gb.tensor, gbo, [[pstep, P], [8, H], [1, HID], [80, NCH]])
                exm_o = exm[:].offset
                exm_p = exm[:].ap[0][0]
                exm_v = AP(exm.tensor, exm_o, [[exm_p, P], [NCH, H], [0, HID], [1, NCH]])
                msgw = pool.tile([P, H, HID, NCH], f32, tag="msgw")
                nc.vector.tensor_tensor(msgw[:], gh_v, exm_v, mybir.AluOpType.mult)
                U = pool.tile([P, H, HID], f32, tag="U")
                nc.vector.tensor_reduce(U[:], msgw[:], mybir.AxisListType.X,
                                        mybir.AluOpType.add)

                rd_v = AP(rd.tensor, rd[:].offset, [[rd[:].ap[0][0], P], [1, H], [0, HID]])
                t2 = pool.tile([P, H * HID], f32, tag="t2")
                nc.vector.tensor_tensor(
                    t2[:].rearrange("p (a b) -> p a b", a=H), U[:], rd_v,
                    mybir.AluOpType.mult)
                t3 = pool.tile([P, H * HID], f32, tag="t3")
                nc.vector.tensor_add(t3[:], t2[:], b1mat[:])
                # elu
                neg = pool.tile([P, H * HID], f32, tag="neg")
                nc.vector.tensor_scalar_min(neg[:], t3[:], 0.0)
                een = pool.tile([P, H * HID], f32, tag="een")
                nc.scalar.activation(een[:], neg[:], mybir.ActivationFunctionType.Exp)
                pos = pool.tile([P, H * HID], f32, tag="pos")
                nc.vector.tensor_scalar_max(pos[:], t3[:], 0.0)
                h1 = pool.tile([P, H * HID], f32, tag="h1")
                nc.vector.scalar_tensor_tensor(
                    out=h1[:], in0=een[:], scalar=-1.0, in1=pos[:],
                    op0=mybir.AluOpType.add, op1=mybir.AluOpType.add)
                # transpose -> O1T[:, g*128:(g+1)*128]
                ps_tr = ppool.tile([H * HID, P], f32, space="PSUM", tag="ptr")
                nc.tensor.transpose(out=ps_tr[:], in_=h1[:], identity=ident[:])
                o1 = pool.tile([H * HID, P], f32, tag="o1")
                nc.vector.tensor_copy(o1[:], ps_tr[:])
                nc.sync.dma_start(t_O1T[:, g * P:(g + 1) * P], o1[:])

            # ---- L2 projection: AGIN rows = [h2' | asrc2 | adst2] ----
            for g in range(G):
                lh = pool.tile([H * HID, P], f32, tag="lh")
                nc.sync.dma_start(lh[:], t_O1T[:, g * P:(g + 1) * P])
                ps2 = ppool.tile([P, 42], f32, space="PSUM", tag="p2")
                nc.tensor.matmul(out=ps2[:], lhsT=lh[:], rhs=w2e[:], start=True, stop=True)
                og = pool.tile([P, 42], f32, tag="og")
                nc.vector.tensor_copy(og[:], ps2[:])
                nc.sync.dma_start(t_AG[g * P:(g + 1) * P, :], og[:])

            _ps.close()

    nc.finalize()
    return nc


def _build_neff2(meta):
    import concourse.bacc as bacc
    import concourse.mybir as mybir
    import concourse.tile as tile
    import concourse.bass as bass

    G, NV, TOT = meta["G"], meta["NV"], meta["TOT"]
    NCHS, CSTART = meta["NCHS"], meta["CSTART"]
    m1_same = meta["m1_same"]
    f32 = mybir.dt.float32
    AP = bass.AP

    nc = bacc.Bacc(num_devices=NCORES)
    t_g2 = nc.dram_tensor("G2", [P, TOT * 42], f32, kind="ExternalInput")
    t_ad2 = nc.dram_tensor("AD2", [P, G], f32, kind="ExternalInput")
    t_b2 = nc.dram_tensor("b2", [1, OUTC], f32, kind="ExternalInput")
    if not m1_same:
        t_M1 = nc.dram_tensor("M1", [P, TOT], f32, kind="ExternalInput")
    t_M2 = nc.dram_tensor("M2", [P, TOT], f32, kind="ExternalInput")
    t_OUT = nc.dram_tensor("OUT2", [NV, OUTC], f32, kind="ExternalOutput")

    with tile.TileContext(nc) as tc:
        with tc.tile_pool(name="const", bufs=1) as cpool, \
             tc.tile_pool(name="work", bufs=3) as pool:

            b2mat = cpool.tile([P, OUTC], f32)
            nc.sync.dma_start(b2mat[:], AP(t_b2, 0, [[0, P], [1, OUTC]]))
            if not m1_same:
                m_m1 = cpool.tile([P, TOT], f32)
                nc.sync.dma_start(m_m1[:], t_M1[:])
            m_m2 = cpool.tile([P, TOT], f32)
            nc.sync.dma_start(m_m2[:], t_M2[:])
            ad2 = cpool.tile([P, G], f32)
            nc.sync.dma_start(ad2[:], t_ad2[:])

            for g in range(G):
                NCH = NCHS[g]
                cs = CSTART[g]
                # halo grid slice [128, NCH, 42]: [h2'(40) | asrc2 | adst2]
                gb = pool.tile([P, NCH, 42], f32, tag="gb")
                nc.sync.dma_start(gb[:].rearrange("p a b -> p (a b)"),
                                  t_g2[:, cs * 42:(cs + NCH) * 42])

                gbo = gb[:].offset
                pstep = gb[:].ap[0][0]
                asrc_v = AP(gb.tensor, gbo + 40, [[pstep, P], [1, 1], [42, NCH]])
                adg = ad2[:, g:g + 1]
                ad_v = AP(ad2.tensor, adg.offset, [[adg.ap[0][0], P], [1, 1], [0, NCH]])
                lg = pool.tile([P, 1, NCH], f32, tag="lg")
                nc.vector.tensor_tensor(lg[:], asrc_v, ad_v, mybir.AluOpType.add)
                lr = pool.tile([P, 1, NCH], f32, tag="lr")
                nc.vector.scalar_tensor_tensor(
                    out=lr[:], in0=lg[:], scalar=SLOPE, in1=lg[:],
                    op0=mybir.AluOpType.mult, op1=mybir.AluOpType.max)
                ex = pool.tile([P, 1, NCH], f32, tag="ex")
                nc.scalar.activation(ex[:], lr[:], mybir.ActivationFunctionType.Exp)

                m2_v = AP(m_m2.tensor, m_m2[:].offset + cs,
                          [[m_m2[:].ap[0][0], P], [0, 1], [1, NCH]])
                exm = pool.tile([P, 1, NCH], f32, tag="exm")
                nc.vector.tensor_tensor(exm[:], ex[:], m2_v, mybir.AluOpType.mult)
                if m1_same:
                    exd = exm
                else:
                    m1_v = AP(m_m1.tensor, m_m1[:].offset + cs,
                              [[m_m1[:].ap[0][0], P], [0, 1], [1, NCH]])
                    exd = pool.tile([P, 1, NCH], f32, tag="exd")
                    nc.vector.tensor_tensor(exd[:], ex[:], m1_v, mybir.AluOpType.mult)

                den = pool.tile([P, 1], f32, tag="den")
                nc.vector.tensor_reduce(den[:], exd[:], mybir.AxisListType.X,
                                        mybir.AluOpType.add)
                den2 = pool.tile([P, 1], f32, tag="den2")
                nc.vector.tensor_scalar_add(den2[:], den[:], 1e-16)
                rd = pool.tile([P, 1], f32, tag="rd")
                nc.vector.reciprocal(rd[:], den2[:])

                gh_v = AP(gb.tensor, gbo, [[pstep, P], [1, OUTC], [42, NCH]])
                exm_v = AP(exm.tensor, exm[:].offset,
                           [[exm[:].ap[0][0], P], [0, OUTC], [1, NCH]])
                msgw = pool.tile([P, OUTC, NCH], f32, tag="msgw")
                nc.vector.tensor_tensor(msgw[:], gh_v, exm_v, mybir.AluOpType.mult)
                U = pool.tile([P, OUTC], f32, tag="U")
                nc.vector.tensor_reduce(U[:], msgw[:], mybir.AxisListType.X,
                                        mybir.AluOpType.add)

                rd_v = AP(rd.tensor, rd[:].offset, [[rd[:].ap[0][0], P], [0, OUTC]])
                t2 = pool.tile([P, OUTC], f32, tag="t2")
                nc.vector.tensor_tensor(t2[:], U[:], rd_v, mybir.AluOpType.mult)
                t3 = pool.tile([P, OUTC], f32, tag="t3")
                nc.vector.tensor_add(t3[:], t2[:], b2mat[:])
                nc.sync.dma_start(t_OUT[g * P:(g + 1) * P, :], t3[:])

    nc.finalize()
    return nc


# --------------------------------------------------------------------------
# host staging (pure layout/sharding, no FLOPs on tensor data)
# --------------------------------------------------------------------------

def _stage_xgrids(x, meta):
    """Per core: halo grid [IN, TOT*P] bf16 (x[src] per grid slot, feature-
    major) and local node features [IN, NV] bf16."""
    xb = np.ascontiguousarray(x).astype(BF16)
    xgs, xns = [], []
    for k in range(NCORES):
        rows = xb[meta["SRCN"][k].reshape(-1)]                # [TOT*P, IN]
        xgs.append(np.ascontiguousarray(rows.T))              # [IN, TOT*P]
        vp = meta["vperm"][k]
        xn = np.zeros((meta["NV"], IN), dtype=BF16)
        valid = vp >= 0
        xn[valid] = xb[vp[valid]]
        xns.append(np.ascontiguousarray(xn.T))                # [IN, NV]
    return xgs, xns


def _stage_grid2(T2full, meta):
    """Per core: L2 halo grid [P, TOT*42] f32 (T2 row per grid slot,
    partition-major) and own adst2 [P, G]."""
    g2s, ad2s = [], []
    NV, G, TOT = meta["NV"], meta["G"], meta["TOT"]
    for k in range(NCORES):
        rows = T2full[meta["SRCROW"][k].reshape(-1)]          # [TOT*P, 42]
        g2 = np.ascontiguousarray(
            rows.reshape(TOT, P, 42).transpose(1, 0, 2).reshape(P, TOT * 42))
        g2s.append(g2)
        own = k * NV + np.arange(NV)
        ad2 = T2full[own, 41].reshape(G, P).T                 # [P, G]
        ad2s.append(np.ascontiguousarray(ad2))
    return g2s, ad2s


# --------------------------------------------------------------------------
# entry point
# --------------------------------------------------------------------------

def kernel(x, edge_index, edge_weight, W1, att_src1, att_dst1, bias1,
           W2, att_src2, att_dst2, bias2):
    SpmdRunner = _inline_runner()

    x = np.asarray(x, dtype=np.float32)
    W1 = np.asarray(W1, dtype=np.float32)
    W2 = np.asarray(W2, dtype=np.float32)
    bias1 = np.asarray(bias1, dtype=np.float32)
    bias2 = np.asarray(bias2, dtype=np.float32)

    import hashlib
    h = hashlib.sha1()
    h.update(np.ascontiguousarray(edge_index).tobytes())
    h.update(np.ascontiguousarray(edge_weight).tobytes())
    key = h.hexdigest()
    if _CACHE.get("key") != key:
        _CACHE.clear()
        _CACHE["key"] = key
        _CACHE["meta"] = _host_prep(edge_index, edge_weight)
    meta = _CACHE["meta"]

    xgs, xns = _stage_xgrids(x, meta)
    A1 = np.concatenate(
        [_blockdiag(np.asarray(att_src1, np.float32)),
         _blockdiag(np.asarray(att_dst1, np.float32))], axis=1)    # [64, 16]
    A2 = np.concatenate(
        [np.asarray(att_src2, np.float32).reshape(OUTC, 1),
         np.asarray(att_dst2, np.float32).reshape(OUTC, 1)], axis=1)  # [40, 2]

    if "nc1" not in _CACHE:
        _CACHE["nc1"] = _build_neff1(meta)
        _CACHE["run1"] = SpmdRunner(_CACHE["nc1"], NCORES)
    run1 = _CACHE["run1"]

    in_maps1 = []
    for k in range(NCORES):
        m = {
            "XG": xgs[k], "XN": xns[k],
            "W1": W1, "A1": A1, "b1": bias1.reshape(1, -1),
            "W2": W2, "A2": A2,
            "M2": np.asarray(meta["M2"][k]),
        }
        if not meta["m1_same"]:
            m["M1"] = np.asarray(meta["M1"][k])
        in_maps1.append(m)
    args1 = run1.prepare(in_maps1)
    _CACHE["args1_cached"] = args1
    res1 = run1.results(run1.run(args1))

    T2full = np.concatenate([r["AGIN"] for r in res1], axis=0)     # [NVG, 42]

    g2s, ad2s = _stage_grid2(T2full, meta)

    if "nc2" not in _CACHE:
        _CACHE["nc2"] = _build_neff2(meta)
        _CACHE["run2"] = SpmdRunner(_CACHE["nc2"], NCORES)
    run2 = _CACHE["run2"]

    in_maps2 = []
    for k in range(NCORES):
        m = {
            "G2": g2s[k], "AD2": ad2s[k], "b2": bias2.reshape(1, -1),
            "M2": np.asarray(meta["M2"][k]),
        }
        if not meta["m1_same"]:
            m["M1"] = np.asarray(meta["M1"][k])
        in_maps2.append(m)
    args2 = run2.prepare(in_maps2)
    _CACHE["args2_cached"] = args2
    res2 = run2.results(run2.run(args2))

    out = np.zeros((N, OUTC), dtype=np.float32)
    for k in range(NCORES):
        vp = meta["vperm"][k]
        valid = vp >= 0
        out[vp[valid]] = res2[k]["OUT2"][np.flatnonzero(valid)]
    return out


def _inline_runner():
    """Self-contained copy of runner.SpmdRunner for harness environments."""
    import time
    import jax
    from jax.sharding import Mesh, PartitionSpec
    from jax.experimental.shard_map import shard_map
    import concourse.mybir as mybir
    from concourse import bass2jax
    from concourse.bass2jax import _bass_exec_p, partition_id_tensor

    class SpmdRunner:
        def __init__(self, nc, n_cores):
            bass2jax.install_neuronx_cc_hook()
            self.nc = nc
            self.n_cores = n_cores
            in_names, out_names, out_avals, zero_outs = [], [], [], []
            partition_name = (nc.partition_id_tensor.name
                              if nc.partition_id_tensor else None)
            for alloc in nc.m.functions[0].allocations:
                if not isinstance(alloc, mybir.MemoryLocationSet):
                    continue
                name = alloc.memorylocations[0].name
                if alloc.kind == "ExternalInput":
                    if name != partition_name:
                        in_names.append(name)
                elif alloc.kind == "ExternalOutput":
                    shape = tuple(alloc.tensor_shape)
                    dtype = mybir.dt.np(alloc.dtype)
                    out_names.append(name)
                    out_avals.append(jax.core.ShapedArray(shape, dtype))
                    zero_outs.append(np.zeros(shape, dtype))
            self.in_names = list(in_names)
            self.out_names, self.out_avals, self.zero_outs = out_names, out_avals, zero_outs
            n_params, n_outs = len(in_names), len(out_avals)
            all_in = in_names + out_names + ([partition_name] if partition_name else [])

            def _body(*args):
                operands = list(args)
                if partition_name is not None:
                    operands.append(partition_id_tensor())
                return tuple(_bass_exec_p.bind(
                    *operands, out_avals=tuple(out_avals), in_names=tuple(all_in),
                    out_names=tuple(out_names), lowering_input_output_aliases=(),
                    sim_require_finite=True, sim_require_nnan=True, nc=nc))

            devices = jax.devices()[:n_cores]
            mesh = Mesh(np.asarray(devices), ("core",))
            in_specs = (PartitionSpec("core"),) * (n_params + n_outs)
            out_specs = (PartitionSpec("core"),) * n_outs
            self.fn = jax.jit(shard_map(_body, mesh=mesh, in_specs=in_specs,
                                        out_specs=out_specs, check_rep=False),
                              keep_unused=True)
            self.n_params, self.n_outs = n_params, n_outs
            self._mesh = mesh

        def prepare(self, in_maps, device_put=True):
            import jax
            from jax.sharding import PartitionSpec
            per_core = [[np.asarray(m[nm]) for nm in self.in_names] for m in in_maps]
            args = [np.concatenate([per_core[c][i] for c in range(self.n_cores)], axis=0)
                    for i in range(self.n_params)]
            args += [np.zeros((self.n_cores * z.shape[0], *z.shape[1:]), z.dtype)
                     for z in self.zero_outs]
            if device_put:
                sh = jax.sharding.NamedSharding(self._mesh, PartitionSpec("core"))
                args = [jax.device_put(a, sh) for a in args]
                jax.block_until_ready(args)
            return args

        def run(self, args):
            import jax
            outs = self.fn(*args)
            jax.block_until_ready(outs)
            return outs

        def results(self, outs):
            return [{nm: np.asarray(outs[i]).reshape(
                        self.n_cores, *self.out_avals[i].shape)[c]
                     for i, nm in enumerate(self.out_names)}
                    for c in range(self.n_cores)]

    return SpmdRunner


# revision 23
# speedup vs baseline: 52.1001x; 50.1079x over previous
"""2-layer GAT on 8 Trainium2 NeuronCores (Bass/Tile).

Strategy (edge-parallel, per the sharding hint: "each device holds a shard
of edge_index/edge_weight plus replicated (or gathered halo) node features"):

- Destination nodes are assigned round-robin by degree rank to the 8 cores;
  per core, dst slots are grouped 128 per partition-group. A group's edges
  live in a [128 partitions = dst slot, NCH columns] grid (self-loop first,
  padded slots masked to zero).
- The host stages each core's edge shard WITH ITS GATHERED HALO: for every
  grid slot, the raw source-node feature vector (bf16) — i.e. x[src] laid
  out in grid order. No indirect/gather DMA is ever issued on device (the
  SWDGE gather path costs ~16us/instruction on this runtime); the device
  streams the halo grid sequentially at full HBM bandwidth.
- ALL math runs on device: L1 projection of every (edge-slot) source vector
  on the PE (x@[W1|W1@A1] per grid column), segment softmax + weighted
  aggregation on DVE/ACT, ELU, L2 projection on PE.
- Between the two layer NEFFs the host re-shards the layer-1 activations
  the same way (gathered halo per edge slot) — replacing the cross-device
  exchange; segment reductions stay core-local because edges are
  dst-partitioned.
"""
import numpy as np
import ml_dtypes

N = 100000
E = 1600000
IN = 128
HID = 8
HEADS = 8
OUTC = 40
SLOPE = 0.2
NCORES = 8
P = 128
PCOLS = 6        # grid columns projected per PSUM tile (6*80 = 480 <= 512)

BF16 = ml_dtypes.bfloat16

_CACHE = {}


# --------------------------------------------------------------------------
# host-side index preprocessing (pure integer/layout work)
# --------------------------------------------------------------------------

def _host_prep(edge_index, edge_weight):
    src = np.asarray(edge_index[0], dtype=np.int64)
    dst = np.asarray(edge_index[1], dtype=np.int64)
    ew = np.asarray(edge_weight, dtype=np.float32)
    n = N

    deg = np.bincount(dst, minlength=n).astype(np.int64)          # real in-degree
    order = np.argsort(-deg, kind="stable")                        # degree desc
    core_of = np.empty(n, dtype=np.int64)
    slot_of = np.empty(n, dtype=np.int64)
    core_of[order] = np.arange(n) % NCORES
    slot_of[order] = np.arange(n) // NCORES                        # rank within core

    n_per_core = [int((core_of == k).sum()) for k in range(NCORES)]
    nmax = max(n_per_core)
    G = (nmax + P - 1) // P
    NV = G * P                                                     # slots per core

    # vperm[k][v] = real node at slot v of core k  (-1 = pad)
    vperm = np.full((NCORES, NV), -1, dtype=np.int64)
    vperm[core_of, slot_of] = np.arange(n)

    rowof = core_of * NV + slot_of                                 # [n] T2 row

    # group schedule: NCH per group = max (deg+1) among its 128 nodes, over cores
    degv = np.zeros((NCORES, NV), dtype=np.int64)
    degv[core_of, slot_of] = deg + 1                               # incl self-loop
    nch_per = degv.reshape(NCORES, G, P).max(axis=2)               # [NCORES, G]
    NCHS = nch_per.max(axis=0).astype(np.int64)                    # [G]
    NCHS = np.maximum(NCHS, 1)
    CSTART = np.concatenate([[0], np.cumsum(NCHS)])                # [G+1]
    TOT = int(CSTART[-1])

    # place edges: edge e of dst d -> (core_of[d], partition slot%P,
    # group slot//P, column 1 + rank-within-node); self-loop at column 0.
    e_core = core_of[dst]
    e_slot = slot_of[dst]
    ordr = np.argsort(e_core * NV + e_slot, kind="stable")
    sc, ss = e_core[ordr], e_slot[ordr]
    key = sc * NV + ss
    first = np.r_[True, key[1:] != key[:-1]]
    start_pos = np.flatnonzero(first)
    runlen = np.arange(len(key)) - np.repeat(start_pos, np.diff(np.r_[start_pos, len(key)]))
    e_col = np.empty(E, dtype=np.int64)
    e_col[ordr] = runlen + 1                                       # col 0 = self loop

    # per-core grid metadata
    SRCN = np.zeros((NCORES, TOT, P), dtype=np.int64)              # source node id
    M2 = np.zeros((NCORES, P, TOT), dtype=np.float32)              # ew * validity
    M1 = np.zeros((NCORES, P, TOT), dtype=np.float32)              # validity

    e_g = e_slot // P
    e_p = e_slot % P
    e_c = CSTART[e_g] + e_col
    SRCN[e_core, e_c, e_p] = src
    M2[e_core, e_p, e_c] = ew
    M1[e_core, e_p, e_c] = 1.0

    # self loops (only for real nodes)
    rn = np.arange(n)
    s_c = CSTART[slot_of // P]
    SRCN[core_of, s_c, slot_of % P] = rn
    M2[core_of, slot_of % P, s_c] = 1.0
    M1[core_of, slot_of % P, s_c] = 1.0

    # L2 grid source rows: rowof[SRCN]; pads point at row 0
    SRCROW = rowof[SRCN]                                           # [NCORES, TOT, P]

    return dict(G=G, NV=NV, NVG=NCORES * NV, NCHS=NCHS.tolist(),
                CSTART=CSTART.tolist(), TOT=TOT,
                m1_same=bool(np.array_equal(M1, M2)),
                vperm=vperm, SRCN=SRCN, SRCROW=SRCROW, M1=M1, M2=M2)


def _blockdiag(att):
    """att [H, C] -> [H*C, H] block diagonal."""
    h, c = att.shape
    A = np.zeros((h * c, h), dtype=np.float32)
    for i in range(h):
        A[i * c:(i + 1) * c, i] = att[i]
    return A


# --------------------------------------------------------------------------
# device program builders
# --------------------------------------------------------------------------

def _build_neff1(meta):
    import concourse.bacc as bacc
    import concourse.mybir as mybir
    import concourse.tile as tile
    import concourse.bass as bass
    from concourse.masks import make_identity

    G, NV, TOT = meta["G"], meta["NV"], meta["TOT"]
    NCHS, CSTART = meta["NCHS"], meta["CSTART"]
    m1_same = meta["m1_same"]
    bf16, f32 = mybir.dt.bfloat16, mybir.dt.float32
    AP = bass.AP
    H = HEADS

    nc = bacc.Bacc(num_devices=NCORES)
    t_xg = nc.dram_tensor("XG", [IN, TOT * P], bf16, kind="ExternalInput")
    t_xn = nc.dram_tensor("XN", [IN, NV], bf16, kind="ExternalInput")
    t_W1 = nc.dram_tensor("W1", [IN, HEADS * HID], f32, kind="ExternalInput")
    t_A1 = nc.dram_tensor("A1", [HEADS * HID, 2 * HEADS], f32, kind="ExternalInput")
    t_b1 = nc.dram_tensor("b1", [1, HEADS * HID], f32, kind="ExternalInput")
    t_W2 = nc.dram_tensor("W2", [HEADS * HID, OUTC], f32, kind="ExternalInput")
    t_A2 = nc.dram_tensor("A2", [OUTC, 2], f32, kind="ExternalInput")
    if not m1_same:
        t_M1 = nc.dram_tensor("M1", [P, TOT], f32, kind="ExternalInput")
    t_M2 = nc.dram_tensor("M2", [P, TOT], f32, kind="ExternalInput")

    t_AG = nc.dram_tensor("AGIN", [NV, 42], f32, kind="ExternalOutput")

    with tile.TileContext(nc) as tc:
        with tc.tile_pool(name="const", bufs=1) as cpool:

            ident = cpool.tile([P, P], f32)
            make_identity(nc, ident[:])

            w1eeb = cpool.tile([IN, 80], bf16)
            w1adb = cpool.tile([IN, H], bf16)
            w2e = cpool.tile([HEADS * HID, 42], f32)
            with tc.tile_pool(name="psetup", bufs=1, space="PSUM") as spp:
                # ---- build [W1 | W1@A1src] bf16 and W1@A1dst bf16 ----
                w1 = cpool.tile([IN, HEADS * HID], f32)
                nc.sync.dma_start(w1[:], t_W1[:])
                a1 = cpool.tile([HEADS * HID, 2 * HEADS], f32)
                nc.sync.dma_start(a1[:], t_A1[:])
                ps_w1t = spp.tile([HEADS * HID, IN], f32, space="PSUM")
                nc.tensor.transpose(out=ps_w1t[:], in_=w1[:], identity=ident[:])
                w1t = cpool.tile([HEADS * HID, IN], f32)
                nc.vector.tensor_copy(w1t[:], ps_w1t[:])
                ps_w1a = spp.tile([IN, 2 * HEADS], f32, space="PSUM")
                nc.tensor.matmul(out=ps_w1a[:], lhsT=w1t[:], rhs=a1[:],
                                 start=True, stop=True)
                nc.vector.tensor_copy(w1eeb[:, 0:64], w1[:])
                nc.vector.tensor_copy(w1eeb[:, 64:72], ps_w1a[:, 0:H])
                nc.vector.tensor_copy(w1eeb[:, 72:80], ps_w1a[:, H:2 * H])
                nc.vector.tensor_copy(w1adb[:], ps_w1a[:, H:2 * H])

                # ---- build W2e [64, 42] f32 = [W2 | W2@A2] ----
                w2 = cpool.tile([HEADS * HID, OUTC], f32)
                nc.sync.dma_start(w2[:], t_W2[:])
                a2 = cpool.tile([OUTC, 2], f32)
                nc.sync.dma_start(a2[:], t_A2[:])
                ps_w2t = spp.tile([OUTC, HEADS * HID], f32, space="PSUM")
                nc.tensor.transpose(out=ps_w2t[:], in_=w2[:],
                                    identity=ident[0:HEADS * HID, 0:HEADS * HID])
                w2t = cpool.tile([OUTC, HEADS * HID], f32)
                nc.vector.tensor_copy(w2t[:], ps_w2t[:])
                ps_w2a = spp.tile([HEADS * HID, 2], f32, space="PSUM")
                nc.tensor.matmul(out=ps_w2a[:], lhsT=w2t[:], rhs=a2[:],
                                 start=True, stop=True)
                nc.vector.tensor_copy(w2e[:, 0:OUTC], w2[:])
                nc.vector.tensor_copy(w2e[:, OUTC:42], ps_w2a[:])

            # ---- bias1 replicated [128, 64] ----
            b1mat = cpool.tile([P, HEADS * HID], f32)
            nc.sync.dma_start(b1mat[:], AP(t_b1, 0, [[0, P], [1, HEADS * HID]]))

            # ---- masks resident in SBUF ----
            if not m1_same:
                m_m1 = cpool.tile([P, TOT], f32)
                nc.sync.dma_start(m_m1[:], t_M1[:])
            m_m2 = cpool.tile([P, TOT], f32)
            nc.sync.dma_start(m_m2[:], t_M2[:])

            from contextlib import ExitStack
            _ps = ExitStack()
            pool = _ps.enter_context(tc.tile_pool(name="work", bufs=3))
            ppool = _ps.enter_context(tc.tile_pool(name="psum", bufs=2, space="PSUM"))

            # ---- per-slot own adst: project local node features ----
            adall = cpool.tile([P, G, H], f32)
            adall_flat = adall[:].rearrange("p a b -> p (a b)")
            ADB = 4
            for g0 in range(0, G, ADB):
                gw = min(ADB, G - g0)
                xn = pool.tile([IN, gw * P], bf16, tag="xn")
                nc.sync.dma_start(xn[:], t_xn[:, g0 * P:(g0 + gw) * P])
                ps_ad = ppool.tile([P, ADB * H], f32, space="PSUM", tag="pad")
                for j in range(gw):
                    nc.tensor.matmul(out=ps_ad[:, j * H:(j + 1) * H],
                                     lhsT=xn[:, j * P:(j + 1) * P], rhs=w1adb[:],
                                     start=True, stop=True)
                nc.scalar.copy(adall_flat[:, g0 * H:(g0 + gw) * H],
                               ps_ad[:, 0:gw * H])
            ado = adall[:].offset
            adp = adall[:].ap[0][0]
            m2o = m_m2[:].offset
            m2p = m_m2[:].ap[0][0]

            # ---- edge phase: supergroups of equal-NCH groups ----
            for (g0, S, NCH) in _sg_runs(NCHS, max_cols=48):
                SN = S * NCH
                cs = CSTART[g0]
                xg = pool.tile([IN, SN * P], bf16, tag="xg")
                nc.sync.dma_start(xg[:], t_xg[:, cs * P:(cs + SN) * P])

                # project halo grid: gb[p, (s,c), 80] = [h | asrc | adst]
                gb = pool.tile([P, SN, 80], bf16, tag="gb")
                gb_flat = gb[:].rearrange("p a b -> p (a b)")
                for c0 in range(0, SN, PCOLS):
                    cw = min(PCOLS, SN - c0)
                    ps = ppool.tile([P, PCOLS * 80], f32, space="PSUM", tag="pj")
                    for j in range(cw):
                        nc.tensor.matmul(
                            out=ps[:, j * 80:(j + 1) * 80],
                            lhsT=xg[:, (c0 + j) * P:(c0 + j + 1) * P],
                            rhs=w1eeb[:], start=True, stop=True)
                    nc.scalar.copy(gb_flat[:, c0 * 80:(c0 + cw) * 80],
                                   ps[:, 0:cw * 80])

                gbo = gb[:].offset
                pstep = gb[:].ap[0][0]
                # l[p,h,s,c] = asrc(by src) + adst(own dst)
                asrc_v = AP(gb.tensor, gbo + 64,
                            [[pstep, P], [1, H], [80 * NCH, S], [80, NCH]])
                ad_v = AP(adall.tensor, ado + g0 * H,
                          [[adp, P], [1, H], [H, S], [0, NCH]])
                lg = pool.tile([P, H, SN], f32, tag="lg")
                lgo = lg[:].offset
                lgp = lg[:].ap[0][0]
                lg_v = AP(lg.tensor, lgo, [[lgp, P], [SN, H], [NCH, S], [1, NCH]])
                nc.vector.tensor_tensor(lg_v, asrc_v, ad_v, mybir.AluOpType.add)
                lg_f = AP(lg.tensor, lgo, [[lgp, P], [1, H * SN]])
                lr = pool.tile([P, H * SN], f32, tag="lr")
                nc.vector.scalar_tensor_tensor(
                    out=lr[:], in0=lg_f, scalar=SLOPE, in1=lg_f,
                    op0=mybir.AluOpType.mult, op1=mybir.AluOpType.max)
                ex = pool.tile([P, H * SN], f32, tag="ex")
                nc.scalar.activation(ex[:], lr[:], mybir.ActivationFunctionType.Exp)
                exo = ex[:].offset
                exp_ = ex[:].ap[0][0]

                m2_v = AP(m_m2.tensor, m2o + cs,
                          [[m2p, P], [0, H], [NCH, S], [1, NCH]])
                ex_v = AP(ex.tensor, exo, [[exp_, P], [SN, H], [NCH, S], [1, NCH]])
                exm = pool.tile([P, H, SN], bf16, tag="exm")
                exmo = exm[:].offset
                exmp = exm[:].ap[0][0]
                exm_4 = AP(exm.tensor, exmo, [[exmp, P], [SN, H], [NCH, S], [1, NCH]])
                nc.vector.tensor_tensor(exm_4, ex_v, m2_v, mybir.AluOpType.mult)
                if m1_same:
                    exd_4 = exm_4
                else:
                    m1_v = AP(m_m1.tensor, m_m1[:].offset + cs,
                              [[m_m1[:].ap[0][0], P], [0, H], [NCH, S], [1, NCH]])
                    exd = pool.tile([P, H, SN], bf16, tag="exd")
                    exd_4 = AP(exd.tensor, exd[:].offset,
                               [[exd[:].ap[0][0], P], [SN, H], [NCH, S], [1, NCH]])
                    nc.vector.tensor_tensor(exd_4, ex_v, m1_v, mybir.AluOpType.mult)

                den = pool.tile([P, H, S], f32, tag="den")
                nc.vector.tensor_reduce(den[:], exd_4, mybir.AxisListType.X,
                                        mybir.AluOpType.add)
                den2 = pool.tile([P, H * S], f32, tag="den2")
                nc.vector.tensor_scalar_add(
                    den2[:], den[:].rearrange("p a b -> p (a b)"), 1e-16)
                rd = pool.tile([P, H, S], f32, tag="rd")
                nc.vector.reciprocal(rd[:].rearrange("p a b -> p (a b)"), den2[:])
                rdo = rd[:].offset
                rdp = rd[:].ap[0][0]

                # msgw[p, h, j, (s,c)] = h_src[p, (s,c), 8h+j] * exm[p, h, (s,c)]
                gh_v = AP(gb.tensor, gbo, [[pstep, P], [8, H], [1, HID], [80, SN]])
                exm_b = AP(exm.tensor, exmo, [[exmp, P], [SN, H], [0, HID], [1, SN]])
                msgw = pool.tile([P, H, HID, SN], bf16, tag="msgw")
                nc.vector.tensor_tensor(msgw[:], gh_v, exm_b, mybir.AluOpType.mult)
                mo = msgw[:].offset
                mp = msgw[:].ap[0][0]

                for s in range(S):
                    g = g0 + s
                    msg_s = AP(msgw.tensor, mo + s * NCH,
                               [[mp, P], [HID * SN, H], [SN, HID], [1, NCH]])
                    U = pool.tile([P, H, HID], f32, tag="U")
                    nc.vector.tensor_reduce(U[:], msg_s, mybir.AxisListType.X,
                                            mybir.AluOpType.add)

                    rd_v = AP(rd.tensor, rdo + s, [[rdp, P], [S, H], [0, HID]])
                    t2 = pool.tile([P, H * HID], f32, tag="t2")
                    nc.vector.tensor_tensor(
                        t2[:].rearrange("p (a b) -> p a b", a=H), U[:], rd_v,
                        mybir.AluOpType.mult)
                    t3 = pool.tile([P, H * HID], f32, tag="t3")
                    nc.vector.tensor_add(t3[:], t2[:], b1mat[:])
                    # elu
                    neg = pool.tile([P, H * HID], f32, tag="neg")
                    nc.vector.tensor_scalar_min(neg[:], t3[:], 0.0)
                    een = pool.tile([P, H * HID], f32, tag="een")
                    nc.scalar.activation(een[:], neg[:],
                                         mybir.ActivationFunctionType.Exp)
                    pos = pool.tile([P, H * HID], f32, tag="pos")
                    nc.vector.tensor_scalar_max(pos[:], t3[:], 0.0)
                    h1 = pool.tile([P, H * HID], f32, tag="h1")
                    nc.vector.scalar_tensor_tensor(
                        out=h1[:], in0=een[:], scalar=-1.0, in1=pos[:],
                        op0=mybir.AluOpType.add, op1=mybir.AluOpType.add)
                    # transpose + L2 projection: AGIN rows = [h2'|asrc2|adst2]
                    ps_tr = ppool.tile([H * HID, P], f32, space="PSUM", tag="ptr")
                    nc.tensor.transpose(out=ps_tr[:], in_=h1[:], identity=ident[:])
                    o1 = pool.tile([H * HID, P], f32, tag="o1")
                    nc.vector.tensor_copy(o1[:], ps_tr[:])
                    ps2 = ppool.tile([P, 42], f32, space="PSUM", tag="p2")
                    nc.tensor.matmul(out=ps2[:], lhsT=o1[:], rhs=w2e[:],
                                     start=True, stop=True)
                    og = pool.tile([P, 42], f32, tag="og")
                    nc.scalar.copy(og[:], ps2[:])
                    nc.sync.dma_start(t_AG[g * P:(g + 1) * P, :], og[:])

            _ps.close()

    nc.finalize()
    return nc


def _sg_runs(NCHS, max_cols=64, max_s=8):
    """Merge consecutive equal-NCH groups into supergroup runs."""
    runs = []
    g, G = 0, len(NCHS)
    while g < G:
        nch = NCHS[g]
        s = 1
        while (g + s < G and NCHS[g + s] == nch
               and (s + 1) * nch <= max_cols and s + 1 <= max_s):
            s += 1
        runs.append((g, s, nch))
        g += s
    return runs


def _build_neff2(meta):
    import concourse.bacc as bacc
    import concourse.mybir as mybir
    import concourse.tile as tile
    import concourse.bass as bass

    G, NV, TOT = meta["G"], meta["NV"], meta["TOT"]
    NCHS, CSTART = meta["NCHS"], meta["CSTART"]
    m1_same = meta["m1_same"]
    bf16, f32 = mybir.dt.bfloat16, mybir.dt.float32
    AP = bass.AP

    nc = bacc.Bacc(num_devices=NCORES)
    t_g2 = nc.dram_tensor("G2", [P, TOT * 42], bf16, kind="ExternalInput")
    t_ad2 = nc.dram_tensor("AD2", [P, G], bf16, kind="ExternalInput")
    t_b2 = nc.dram_tensor("b2", [1, OUTC], f32, kind="ExternalInput")
    if not m1_same:
        t_M1 = nc.dram_tensor("M1", [P, TOT], f32, kind="ExternalInput")
    t_M2 = nc.dram_tensor("M2", [P, TOT], f32, kind="ExternalInput")
    t_OUT = nc.dram_tensor("OUT2", [NV, OUTC], f32, kind="ExternalOutput")

    with tile.TileContext(nc) as tc:
        with tc.tile_pool(name="const", bufs=1) as cpool, \
             tc.tile_pool(name="work", bufs=3) as pool:

            b2mat = cpool.tile([P, OUTC], f32)
            nc.sync.dma_start(b2mat[:], AP(t_b2, 0, [[0, P], [1, OUTC]]))
            if not m1_same:
                m_m1 = cpool.tile([P, TOT], f32)
                nc.sync.dma_start(m_m1[:], t_M1[:])
            m_m2 = cpool.tile([P, TOT], f32)
            nc.sync.dma_start(m_m2[:], t_M2[:])
            ad2 = cpool.tile([P, G], bf16)
            nc.sync.dma_start(ad2[:], t_ad2[:])
            ad2o = ad2[:].offset
            ad2p = ad2[:].ap[0][0]
            m2o = m_m2[:].offset
            m2p = m_m2[:].ap[0][0]

            for (g0, S, NCH) in _sg_runs(NCHS):
                SN = S * NCH
                cs = CSTART[g0]
                # halo grid slice [128, S*NCH, 42] bf16: [h2'(40)|asrc2|adst2]
                gb = pool.tile([P, SN, 42], bf16, tag="gb")
                nc.sync.dma_start(gb[:].rearrange("p a b -> p (a b)"),
                                  t_g2[:, cs * 42:(cs + SN) * 42])
                gbo = gb[:].offset
                pstep = gb[:].ap[0][0]

                # logits: asrc(by src) + adst(own dst), [P, S, NCH]
                asrc_v = AP(gb.tensor, gbo + 40,
                            [[pstep, P], [42 * NCH, S], [42, NCH]])
                ad_v = AP(ad2.tensor, ad2o + g0, [[ad2p, P], [1, S], [0, NCH]])
                lg = pool.tile([P, S, NCH], f32, tag="lg")
                nc.vector.tensor_tensor(lg[:], asrc_v, ad_v, mybir.AluOpType.add)
                lr = pool.tile([P, SN], f32, tag="lr")
                nc.vector.scalar_tensor_tensor(
                    out=lr[:], in0=lg[:].rearrange("p a b -> p (a b)"),
                    scalar=SLOPE, in1=lg[:].rearrange("p a b -> p (a b)"),
                    op0=mybir.AluOpType.mult, op1=mybir.AluOpType.max)
                ex = pool.tile([P, SN], f32, tag="ex")
                nc.scalar.activation(ex[:], lr[:], mybir.ActivationFunctionType.Exp)

                m2_v = AP(m_m2.tensor, m2o + cs, [[m2p, P], [1, SN]])
                exm = pool.tile([P, SN], bf16, tag="exm")
                nc.vector.tensor_tensor(exm[:], ex[:], m2_v, mybir.AluOpType.mult)
                if m1_same:
                    exd = exm[:]
                else:
                    m1_v = AP(m_m1.tensor, m_m1[:].offset + cs, [[m_m1[:].ap[0][0], P], [1, SN]])
                    exd_t = pool.tile([P, SN], bf16, tag="exd")
                    nc.vector.tensor_tensor(exd_t[:], ex[:], m1_v, mybir.AluOpType.mult)
                    exd = exd_t[:]

                den = pool.tile([P, S], f32, tag="den")
                nc.vector.tensor_reduce(
                    den[:],
                    AP(exd.tensor, exd.offset, [[exd.ap[0][0], P], [NCH, S], [1, NCH]]),
                    mybir.AxisListType.X, mybir.AluOpType.add)
                den2 = pool.tile([P, S], f32, tag="den2")
                nc.vector.tensor_scalar_add(den2[:], den[:], 1e-16)
                rd = pool.tile([P, S], f32, tag="rd")
                nc.vector.reciprocal(rd[:], den2[:])

                # msgw[p, o, (s,c)] = h2[p, (s,c), o] * exm[p, (s,c)]
                gh_v = AP(gb.tensor, gbo, [[pstep, P], [1, OUTC], [42, SN]])
                exm_v = AP(exm.tensor, exm[:].offset,
                           [[exm[:].ap[0][0], P], [0, OUTC], [1, SN]])
                msgw = pool.tile([P, OUTC, SN], bf16, tag="msgw")
                nc.vector.tensor_tensor(msgw[:], gh_v, exm_v, mybir.AluOpType.mult)
                # U[p, o, s] = sum_c msgw — one 4-dim reduce for the whole run
                mo = msgw[:].offset
                mp = msgw[:].ap[0][0]
                U = pool.tile([P, OUTC, S], f32, tag="U")
                nc.vector.tensor_reduce(
                    U[:], AP(msgw.tensor, mo, [[mp, P], [SN, OUTC], [NCH, S], [1, NCH]]),
                    mybir.AxisListType.X, mybir.AluOpType.add)

                # out[p, s, o] = U * rd + b2  (tiles laid [P, S, OUTC] for DMA)
                t2 = pool.tile([P, S, OUTC], f32, tag="t2")
                t2o = t2[:].offset
                t2p = t2[:].ap[0][0]
                t2_osv = AP(t2.tensor, t2o, [[t2p, P], [1, OUTC], [OUTC, S]])
                rd_v = AP(rd.tensor, rd[:].offset,
                          [[rd[:].ap[0][0], P], [0, OUTC], [1, S]])
                nc.vector.tensor_tensor(t2_osv, U[:], rd_v, mybir.AluOpType.mult)
                t3 = pool.tile([P, S, OUTC], f32, tag="t3")
                b2_v = AP(b2mat.tensor, b2mat[:].offset,
                          [[b2mat[:].ap[0][0], P], [0, S], [1, OUTC]])
                nc.vector.tensor_tensor(t3[:], t2[:], b2_v, mybir.AluOpType.add)
                # rows (g0+s)*128 + p of OUT2
                dst = AP(t_OUT, g0 * P * OUTC,
                         [[OUTC, P], [P * OUTC, S], [1, OUTC]])
                nc.sync.dma_start(dst, t3[:])

    nc.finalize()
    return nc


# --------------------------------------------------------------------------
# host staging (pure layout/sharding, no FLOPs on tensor data)
# --------------------------------------------------------------------------

def _stage_xgrids(x, meta):
    """Per core: halo grid [IN, TOT*P] bf16 (x[src] per grid slot, feature-
    major) and local node features [IN, NV] bf16."""
    xb = np.ascontiguousarray(x).astype(BF16)
    xgs, xns = [], []
    for k in range(NCORES):
        rows = xb[meta["SRCN"][k].reshape(-1)]                # [TOT*P, IN]
        xgs.append(np.ascontiguousarray(rows.T))              # [IN, TOT*P]
        vp = meta["vperm"][k]
        xn = np.zeros((meta["NV"], IN), dtype=BF16)
        valid = vp >= 0
        xn[valid] = xb[vp[valid]]
        xns.append(np.ascontiguousarray(xn.T))                # [IN, NV]
    return xgs, xns


def _stage_grid2(T2full, meta):
    """Per core: L2 halo grid [P, TOT*42] bf16 (T2 row per grid slot,
    partition-major) and own adst2 [P, G] bf16."""
    g2s, ad2s = [], []
    NV, G, TOT = meta["NV"], meta["G"], meta["TOT"]
    T2b = T2full.astype(BF16)
    for k in range(NCORES):
        rows = T2b[meta["SRCROW"][k].reshape(-1)]             # [TOT*P, 42]
        g2 = np.ascontiguousarray(
            rows.reshape(TOT, P, 42).transpose(1, 0, 2).reshape(P, TOT * 42))
        g2s.append(g2)
        own = k * NV + np.arange(NV)
        ad2 = T2b[own, 41].reshape(G, P).T                    # [P, G]
        ad2s.append(np.ascontiguousarray(ad2))
    return g2s, ad2s


# --------------------------------------------------------------------------
# entry point
# --------------------------------------------------------------------------

def kernel(x, edge_index, edge_weight, W1, att_src1, att_dst1, bias1,
           W2, att_src2, att_dst2, bias2):
    SpmdRunner = _inline_runner()

    x = np.asarray(x, dtype=np.float32)
    W1 = np.asarray(W1, dtype=np.float32)
    W2 = np.asarray(W2, dtype=np.float32)
    bias1 = np.asarray(bias1, dtype=np.float32)
    bias2 = np.asarray(bias2, dtype=np.float32)

    import hashlib
    h = hashlib.sha1()
    h.update(np.ascontiguousarray(edge_index).tobytes())
    h.update(np.ascontiguousarray(edge_weight).tobytes())
    key = h.hexdigest()
    if _CACHE.get("key") != key:
        _CACHE.clear()
        _CACHE["key"] = key
        _CACHE["meta"] = _host_prep(edge_index, edge_weight)
    meta = _CACHE["meta"]

    xgs, xns = _stage_xgrids(x, meta)
    A1 = np.concatenate(
        [_blockdiag(np.asarray(att_src1, np.float32)),
         _blockdiag(np.asarray(att_dst1, np.float32))], axis=1)    # [64, 16]
    A2 = np.concatenate(
        [np.asarray(att_src2, np.float32).reshape(OUTC, 1),
         np.asarray(att_dst2, np.float32).reshape(OUTC, 1)], axis=1)  # [40, 2]

    if "nc1" not in _CACHE:
        _CACHE["nc1"] = _build_neff1(meta)
        _CACHE["run1"] = SpmdRunner(_CACHE["nc1"], NCORES)
    run1 = _CACHE["run1"]

    in_maps1 = []
    for k in range(NCORES):
        m = {
            "XG": xgs[k], "XN": xns[k],
            "W1": W1, "A1": A1, "b1": bias1.reshape(1, -1),
            "W2": W2, "A2": A2,
            "M2": np.asarray(meta["M2"][k]),
        }
        if not meta["m1_same"]:
            m["M1"] = np.asarray(meta["M1"][k])
        in_maps1.append(m)
    args1 = run1.prepare(in_maps1)
    _CACHE["args1_cached"] = args1
    res1 = run1.results(run1.run(args1))

    T2full = np.concatenate([r["AGIN"] for r in res1], axis=0)     # [NVG, 42]

    g2s, ad2s = _stage_grid2(T2full, meta)

    if "nc2" not in _CACHE:
        _CACHE["nc2"] = _build_neff2(meta)
        _CACHE["run2"] = SpmdRunner(_CACHE["nc2"], NCORES)
    run2 = _CACHE["run2"]

    in_maps2 = []
    for k in range(NCORES):
        m = {
            "G2": g2s[k], "AD2": ad2s[k], "b2": bias2.reshape(1, -1),
            "M2": np.asarray(meta["M2"][k]),
        }
        if not meta["m1_same"]:
            m["M1"] = np.asarray(meta["M1"][k])
        in_maps2.append(m)
    args2 = run2.prepare(in_maps2)
    _CACHE["args2_cached"] = args2
    res2 = run2.results(run2.run(args2))

    out = np.zeros((N, OUTC), dtype=np.float32)
    for k in range(NCORES):
        vp = meta["vperm"][k]
        valid = vp >= 0
        out[vp[valid]] = res2[k]["OUT2"][np.flatnonzero(valid)]
    return out


def _inline_runner():
    """Self-contained copy of runner.SpmdRunner for harness environments."""
    import time
    import jax
    from jax.sharding import Mesh, PartitionSpec
    from jax.experimental.shard_map import shard_map
    import concourse.mybir as mybir
    from concourse import bass2jax
    from concourse.bass2jax import _bass_exec_p, partition_id_tensor

    class SpmdRunner:
        def __init__(self, nc, n_cores):
            bass2jax.install_neuronx_cc_hook()
            self.nc = nc
            self.n_cores = n_cores
            in_names, out_names, out_avals, zero_outs = [], [], [], []
            partition_name = (nc.partition_id_tensor.name
                              if nc.partition_id_tensor else None)
            for alloc in nc.m.functions[0].allocations:
                if not isinstance(alloc, mybir.MemoryLocationSet):
                    continue
                name = alloc.memorylocations[0].name
                if alloc.kind == "ExternalInput":
                    if name != partition_name:
                        in_names.append(name)
                elif alloc.kind == "ExternalOutput":
                    shape = tuple(alloc.tensor_shape)
                    dtype = mybir.dt.np(alloc.dtype)
                    out_names.append(name)
                    out_avals.append(jax.core.ShapedArray(shape, dtype))
                    zero_outs.append(np.zeros(shape, dtype))
            self.in_names = list(in_names)
            self.out_names, self.out_avals, self.zero_outs = out_names, out_avals, zero_outs
            n_params, n_outs = len(in_names), len(out_avals)
            all_in = in_names + out_names + ([partition_name] if partition_name else [])

            def _body(*args):
                operands = list(args)
                if partition_name is not None:
                    operands.append(partition_id_tensor())
                return tuple(_bass_exec_p.bind(
                    *operands, out_avals=tuple(out_avals), in_names=tuple(all_in),
                    out_names=tuple(out_names), lowering_input_output_aliases=(),
                    sim_require_finite=True, sim_require_nnan=True, nc=nc))

            devices = jax.devices()[:n_cores]
            mesh = Mesh(np.asarray(devices), ("core",))
            in_specs = (PartitionSpec("core"),) * (n_params + n_outs)
            out_specs = (PartitionSpec("core"),) * n_outs
            self.fn = jax.jit(shard_map(_body, mesh=mesh, in_specs=in_specs,
                                        out_specs=out_specs, check_rep=False),
                              keep_unused=True)
            self.n_params, self.n_outs = n_params, n_outs
            self._mesh = mesh

        def prepare(self, in_maps, device_put=True):
            import jax
            from jax.sharding import PartitionSpec
            per_core = [[np.asarray(m[nm]) for nm in self.in_names] for m in in_maps]
            args = [np.concatenate([per_core[c][i] for c in range(self.n_cores)], axis=0)
                    for i in range(self.n_params)]
            args += [np.zeros((self.n_cores * z.shape[0], *z.shape[1:]), z.dtype)
                     for z in self.zero_outs]
            if device_put:
                sh = jax.sharding.NamedSharding(self._mesh, PartitionSpec("core"))
                args = [jax.device_put(a, sh) for a in args]
                jax.block_until_ready(args)
            return args

        def run(self, args):
            import jax
            outs = self.fn(*args)
            jax.block_until_ready(outs)
            return outs

        def results(self, outs):
            return [{nm: np.asarray(outs[i]).reshape(
                        self.n_cores, *self.out_avals[i].shape)[c]
                     for i, nm in enumerate(self.out_names)}
                    for c in range(self.n_cores)]

    return SpmdRunner
